# revision 65
# baseline (speedup 1.0000x reference)
"""HFreqC kernel, y^T orientation: B stationary, rows streaming.

Same parity math as kernel.py: y_even = relu(x_even/2 + x_odd@B1), y_odd
symmetric. Here the output is computed TRANSPOSED (channels on partitions,
rows on the free axis), which removes the PE seed matmuls entirely: the
x/2 addend is the channel-major input already in SBUF, applied by DVE.
2x is folded into B and 0.5 into the relu scale: y = relu(0.5*(x + x@2B)).

Per group of R=368 rows (16 uniform groups/core):
  - one [128, 6*R] bf16 channel-major in-DMA (SP): blocks 0-2 odd chs,
    3-5 even chs (parity chunks of 128, third chunk 108+pad20).
  - per (half, je-chunk): 3 matmuls, lhsT = 2B chunk [128ch, 128je],
    rhs = xt block [128, R], accumulating [128, R] f32 in PSUM (1 bank).
  - DVE tensor_tensor add: psum + same-parity xt block -> ysb bf16.
  - one ScalarE relu over the whole [128, 6*R] ysb with scale=0.5.
  - one [128, 6*R] out-DMA per group, alternating SP/ScalarE queues.
Model: PE 288 matmuls 44.2us, DVE 48.8us (ceiling), ACT 34+14us, SP 41us.
"""

import numpy as np

C = 728
H = C // 2          # 364
KT = 3
N_CORES = 8
ROWS_TOTAL = 32 * 38 * 38
ROWS_PER_CORE = 5888
R = 368             # rows per group
N_GROUPS = ROWS_PER_CORE // R   # 16
GW = 6 * R          # free-width per group in xt/ysb (2208)

_CACHE = {}


def _f32_to_bf16_u16(a: np.ndarray) -> np.ndarray:
    u = np.ascontiguousarray(a).view(np.uint32)
    rounded = u + np.uint32(0x7FFF) + ((u >> np.uint32(16)) & np.uint32(1))
    return (rounded >> np.uint32(16)).astype(np.uint16)


def _bf16_u16_to_f32(u: np.ndarray) -> np.ndarray:
    return (u.astype(np.uint32) << np.uint32(16)).view(np.float32)


def _bf16(a: np.ndarray):
    import ml_dtypes
    return _f32_to_bf16_u16(np.ascontiguousarray(a)).view(ml_dtypes.bfloat16)


def _build_w(scale: int) -> np.ndarray:
    m_sh = np.ones(C)
    m_sh[C // 2 - C // scale: C // 2 + C // scale] = 0
    m = np.fft.ifftshift(m_sh)
    A = np.fft.ifft(m[:, None] * np.fft.fft(np.eye(C), axis=0), axis=0)
    return np.real(A).T.astype(np.float32)


def _build_w_chunks(scale: int) -> np.ndarray:
    """[18, 128, 128] bf16: chunk h*9+u*3+jt = (2*B_h)[u-chunk, jt-chunk],
    zero-padded to 128x128. B_0 = W[odd, even], B_1 = W[even, odd]."""
    W = _build_w(scale)
    assert np.abs(W[0::2, 0::2] - 0.5 * np.eye(H)).max() < 1e-5
    assert np.abs(W[1::2, 1::2] - 0.5 * np.eye(H)).max() < 1e-5
    Bs = [2.0 * W[1::2, 0::2], 2.0 * W[0::2, 1::2]]
    out = np.zeros((18, 128, 128), dtype=np.float32)
    for h in range(2):
        Bp = np.zeros((384, 384), dtype=np.float32)
        Bp[:H, :H] = Bs[h]
        for u in range(KT):
            for jt in range(KT):
                out[h * 9 + u * 3 + jt] = \
                    Bp[u * 128:(u + 1) * 128, jt * 128:(jt + 1) * 128]
    # device holds W as one [128, 18*128] tile: [p][i*128+m] = chunk_i[p, m]
    return _bf16(out.transpose(1, 0, 2).reshape(128, 18 * 128))


def _shard_xt(x16: np.ndarray, core: int) -> np.ndarray:
    """[128, N_GROUPS*GW] bf16: [p][g*GW + u*R + r] = x[g*R+r, ch(u,p)],
    u 0-2 odd channels (2*(u*128+p)+1), u 3-5 even (2*((u-3)*128+p))."""
    import ml_dtypes
    lo = core * ROWS_PER_CORE
    hi = min(lo + ROWS_PER_CORE, ROWS_TOTAL)
    xp = np.zeros((ROWS_PER_CORE, 768), dtype=np.uint16)
    xp[:hi - lo, :H] = x16[lo:hi, 1::2]                 # odd channels
    xp[:hi - lo, 384:384 + H] = x16[lo:hi, 0::2]        # even channels
    v = xp.reshape(N_GROUPS, R, 6, 128)                 # g r u p
    v = v.transpose(3, 0, 2, 1)                         # p g u r
    return np.ascontiguousarray(v).reshape(128, N_GROUPS * GW).view(
        ml_dtypes.bfloat16)


def _build_nc(repeat: int = 1, passes_per_iter: int = 1):
    import concourse.mybir as mybir
    import concourse.tile as tile
    from concourse import bacc

    fp32 = mybir.dt.float32
    bf16 = mybir.dt.bfloat16

    nc = bacc.Bacc("TRN2", target_bir_lowering=False)
    x_d = nc.dram_tensor("x", [128, N_GROUPS * GW], bf16,
                         kind="ExternalInput").ap()
    w_d = nc.dram_tensor("w", [128, 18 * 128], bf16, kind="ExternalInput").ap()
    y_d = nc.dram_tensor("y", [128, N_GROUPS * GW], bf16,
                         kind="ExternalOutput").ap()

    with tile.TileContext(nc) as tc:
        with (
            tc.tile_pool(name="wpool", bufs=1) as wpool,
            tc.tile_pool(name="io", bufs=6) as io,
            tc.tile_pool(name="psp", bufs=1, space="PSUM") as psp,
        ):
            w_sb = wpool.tile([128, 18 * 128], bf16, name="w_sb")
            nc.scalar.dma_start(out=w_sb, in_=w_d)
            w_tiles = [w_sb[:, i * 128:(i + 1) * 128] for i in range(18)]

            def one_pass():
                for g in range(N_GROUPS):
                    o = g * GW
                    xt = io.tile([128, GW], bf16, tag="xt")
                    nc.sync.dma_start(out=xt, in_=x_d[:, o:o + GW])
                    ysb = io.tile([128, GW], bf16, tag="y")
                    for half in range(2):
                        for jt in range(KT):
                            v = half * 3 + jt
                            ps = psp.tile([128, R], fp32, name=f"ps{v}",
                                          tag=f"ps{v}",
                                          bufs=2 if jt == 0 else 1)
                            for u in range(KT):
                                # y_even contracts odd blocks (0-2),
                                # y_odd contracts even blocks (3-5)
                                blk = u if half == 0 else 3 + u
                                nc.tensor.matmul(
                                    ps,
                                    lhsT=w_tiles[half * 9 + u * 3 + jt],
                                    rhs=xt[:, blk * R:(blk + 1) * R],
                                    start=(u == 0),
                                    stop=(u == KT - 1),
                                )
                            # + x_same_parity (chunk jt): even blocks for
                            # y_even, odd blocks for y_odd
                            sb = (3 + jt) if half == 0 else jt
                            nc.vector.tensor_tensor(
                                ysb[:, v * R:(v + 1) * R], ps,
                                xt[:, sb * R:(sb + 1) * R],
                                mybir.AluOpType.add)
                    # y = relu(0.5 * (x + x@2B)); two half-width ops so the
                    # first starts after the first 3 adds
                    # split relu+out per half so out of half 0 overlaps the
                    # half-1 relu (shrinks the last-group drain tail)
                    nc.scalar.activation(
                        ysb[:, :3 * R], ysb[:, :3 * R],
                        mybir.ActivationFunctionType.Relu, scale=0.5)
                    nc.gpsimd.dma_start(out=y_d[:, o:o + 3 * R],
                                        in_=ysb[:, :3 * R])
                    nc.scalar.activation(
                        ysb[:, 3 * R:], ysb[:, 3 * R:],
                        mybir.ActivationFunctionType.Relu, scale=0.5)
                    nc.sync.dma_start(out=y_d[:, o + 3 * R:o + GW],
                                      in_=ysb[:, 3 * R:])

            if repeat == 1:
                one_pass()
            else:
                import concourse.mybir as _mb
                with tc.For_i(0, repeat, 1,
                              hint_engines=(_mb.EngineType.PE,),
                              staggered_reset=True):
                    for _ in range(passes_per_iter):
                        one_pass()
    nc.compile()
    return nc


def _make_in_maps(x: np.ndarray, scale: int):
    xf = np.ascontiguousarray(np.asarray(x, dtype=np.float32).reshape(-1, C))
    x16 = _f32_to_bf16_u16(xf)
    W = _build_w_chunks(scale)
    return [{"x": _shard_xt(x16, i), "w": W} for i in range(N_CORES)]


def _unshard_y(yb: np.ndarray, nrows: int) -> np.ndarray:
    """[128, N_GROUPS*GW] bf16-u16 device layout -> [nrows, C] f32."""
    v = yb.reshape(128, N_GROUPS, 6, R)         # p g v r
    v = v.transpose(1, 3, 2, 0)                 # g r v p -> rows, (v, p)
    v = v.reshape(ROWS_PER_CORE, 6 * 128)
    yf = _bf16_u16_to_f32(np.ascontiguousarray(v).view(np.uint16))
    out = np.empty((ROWS_PER_CORE, C), dtype=np.float32)
    # v blocks 0-2: even channels 2*(v*128+p); 3-5: odd channels
    out[:, 0::2] = yf[:, :H]
    out[:, 1::2] = yf[:, 384:384 + H]
    return out[:nrows]


def kernel(x: np.ndarray, scale) -> np.ndarray:
    import sys
    if "/opt/trn_rl_repo" not in sys.path:
        sys.path.insert(0, "/opt/trn_rl_repo")
    from concourse.bass_utils import run_bass_kernel_spmd

    scale = int(np.asarray(scale))
    x = np.asarray(x, dtype=np.float32)
    orig_shape = x.shape

    if "nc" not in _CACHE:
        _CACHE["nc"] = _build_nc()
    nc = _CACHE["nc"]

    in_maps = _make_in_maps(x, scale)
    res = run_bass_kernel_spmd(nc, in_maps, list(range(N_CORES)))
    outs = []
    for i, r in enumerate(res.results):
        lo = i * ROWS_PER_CORE
        hi = min(lo + ROWS_PER_CORE, ROWS_TOTAL)
        outs.append(_unshard_y(np.asarray(r["y"]), hi - lo))
    y = np.concatenate(outs, axis=0).reshape(orig_shape)
    return y.astype(np.float32)


# revision 66
# speedup vs baseline: 1.0469x; 1.0469x over previous
"""HFreqC kernel, y^T orientation: B stationary, rows streaming.

Same parity math as kernel.py: y_even = relu(x_even/2 + x_odd@B1), y_odd
symmetric. Here the output is computed TRANSPOSED (channels on partitions,
rows on the free axis), which removes the PE seed matmuls entirely: the
x/2 addend is the channel-major input already in SBUF, applied by DVE.
2x is folded into B and 0.5 into the relu scale: y = relu(0.5*(x + x@2B)).

Per group of R=368 rows (16 uniform groups/core):
  - one [128, 6*R] bf16 channel-major in-DMA (SP): blocks 0-2 odd chs,
    3-5 even chs (parity chunks of 128, third chunk 108+pad20).
  - per (half, je-chunk): 3 matmuls, lhsT = 2B chunk [128ch, 128je],
    rhs = xt block [128, R], accumulating [128, R] f32 in PSUM (1 bank).
  - DVE tensor_tensor add: psum + same-parity xt block -> ysb bf16.
  - one ScalarE relu over the whole [128, 6*R] ysb with scale=0.5.
  - one [128, 6*R] out-DMA per group, alternating SP/ScalarE queues.
Model: PE 288 matmuls 44.2us, DVE 48.8us (ceiling), ACT 34+14us, SP 41us.
"""

import numpy as np

C = 728
H = C // 2          # 364
KT = 3
N_CORES = 8
ROWS_TOTAL = 32 * 38 * 38
ROWS_PER_CORE = 5888
R = 368             # rows per group
N_GROUPS = ROWS_PER_CORE // R   # 16
GW = 6 * R          # free-width per group in xt/ysb (2208)

_CACHE = {}


def _f32_to_bf16_u16(a: np.ndarray) -> np.ndarray:
    u = np.ascontiguousarray(a).view(np.uint32)
    rounded = u + np.uint32(0x7FFF) + ((u >> np.uint32(16)) & np.uint32(1))
    return (rounded >> np.uint32(16)).astype(np.uint16)


def _bf16_u16_to_f32(u: np.ndarray) -> np.ndarray:
    return (u.astype(np.uint32) << np.uint32(16)).view(np.float32)


def _bf16(a: np.ndarray):
    import ml_dtypes
    return _f32_to_bf16_u16(np.ascontiguousarray(a)).view(ml_dtypes.bfloat16)


def _build_w(scale: int) -> np.ndarray:
    m_sh = np.ones(C)
    m_sh[C // 2 - C // scale: C // 2 + C // scale] = 0
    m = np.fft.ifftshift(m_sh)
    A = np.fft.ifft(m[:, None] * np.fft.fft(np.eye(C), axis=0), axis=0)
    return np.real(A).T.astype(np.float32)


def _build_w_chunks(scale: int) -> np.ndarray:
    """[18, 128, 128] bf16: chunk h*9+u*3+jt = (2*B_h)[u-chunk, jt-chunk],
    zero-padded to 128x128. B_0 = W[odd, even], B_1 = W[even, odd]."""
    W = _build_w(scale)
    assert np.abs(W[0::2, 0::2] - 0.5 * np.eye(H)).max() < 1e-5
    assert np.abs(W[1::2, 1::2] - 0.5 * np.eye(H)).max() < 1e-5
    Bs = [2.0 * W[1::2, 0::2], 2.0 * W[0::2, 1::2]]
    out = np.zeros((18, 128, 128), dtype=np.float32)
    for h in range(2):
        Bp = np.zeros((384, 384), dtype=np.float32)
        Bp[:H, :H] = Bs[h]
        for u in range(KT):
            for jt in range(KT):
                out[h * 9 + u * 3 + jt] = \
                    Bp[u * 128:(u + 1) * 128, jt * 128:(jt + 1) * 128]
    # device holds W as one [128, 18*128] tile: [p][i*128+m] = chunk_i[p, m]
    return _bf16(out.transpose(1, 0, 2).reshape(128, 18 * 128))


def _shard_xt(x16: np.ndarray, core: int) -> np.ndarray:
    """[128, N_GROUPS*GW] bf16: [p][g*GW + u*R + r] = x[g*R+r, ch(u,p)],
    u 0-2 odd channels (2*(u*128+p)+1), u 3-5 even (2*((u-3)*128+p))."""
    import ml_dtypes
    lo = core * ROWS_PER_CORE
    hi = min(lo + ROWS_PER_CORE, ROWS_TOTAL)
    xp = np.zeros((ROWS_PER_CORE, 768), dtype=np.uint16)
    xp[:hi - lo, :H] = x16[lo:hi, 1::2]                 # odd channels
    xp[:hi - lo, 384:384 + H] = x16[lo:hi, 0::2]        # even channels
    v = xp.reshape(N_GROUPS, R, 6, 128)                 # g r u p
    v = v.transpose(3, 0, 2, 1)                         # p g u r
    return np.ascontiguousarray(v).reshape(128, N_GROUPS * GW).view(
        ml_dtypes.bfloat16)


def _build_nc(repeat: int = 1, passes_per_iter: int = 1):
    import concourse.mybir as mybir
    import concourse.tile as tile
    from concourse import bacc

    fp32 = mybir.dt.float32
    bf16 = mybir.dt.bfloat16

    nc = bacc.Bacc("TRN2", target_bir_lowering=False)
    x_d = nc.dram_tensor("x", [128, N_GROUPS * GW], bf16,
                         kind="ExternalInput").ap()
    w_d = nc.dram_tensor("w", [128, 18 * 128], bf16, kind="ExternalInput").ap()
    y_d = nc.dram_tensor("y", [128, N_GROUPS * GW], bf16,
                         kind="ExternalOutput").ap()

    with tile.TileContext(nc) as tc:
        with (
            tc.tile_pool(name="wpool", bufs=1) as wpool,
            tc.tile_pool(name="io", bufs=6) as io,
            tc.tile_pool(name="psp", bufs=1, space="PSUM") as psp,
        ):
            w_sb = wpool.tile([128, 18 * 128], bf16, name="w_sb")
            nc.scalar.dma_start(out=w_sb, in_=w_d)
            w_tiles = [w_sb[:, i * 128:(i + 1) * 128] for i in range(18)]

            def one_pass():
                for g in range(N_GROUPS):
                    o = g * GW
                    xt = io.tile([128, GW], bf16, tag="xt")
                    nc.sync.dma_start(out=xt, in_=x_d[:, o:o + GW])
                    ysb = io.tile([128, GW], bf16, tag="y")
                    for half in range(2):
                        for jt in range(KT):
                            v = half * 3 + jt
                            ps = psp.tile([128, R], fp32, name=f"ps{v}",
                                          tag=f"ps{v}", bufs=1)
                            for u in range(KT):
                                # y_even contracts odd blocks (0-2),
                                # y_odd contracts even blocks (3-5)
                                blk = u if half == 0 else 3 + u
                                nc.tensor.matmul(
                                    ps,
                                    lhsT=w_tiles[half * 9 + u * 3 + jt],
                                    rhs=xt[:, blk * R:(blk + 1) * R],
                                    start=(u == 0),
                                    stop=(u == KT - 1),
                                )
                            # + x_same_parity (chunk jt): even blocks for
                            # y_even, odd blocks for y_odd
                            sb = (3 + jt) if half == 0 else jt
                            nc.vector.tensor_tensor(
                                ysb[:, v * R:(v + 1) * R], ps,
                                xt[:, sb * R:(sb + 1) * R],
                                mybir.AluOpType.add)
                    # y = relu(0.5 * (x + x@2B)); two half-width ops so the
                    # first starts after the first 3 adds
                    nc.scalar.activation(
                        ysb[:, :3 * R], ysb[:, :3 * R],
                        mybir.ActivationFunctionType.Relu, scale=0.5)
                    nc.scalar.activation(
                        ysb[:, 3 * R:], ysb[:, 3 * R:],
                        mybir.ActivationFunctionType.Relu, scale=0.5)
                    out_eng = nc.gpsimd if g % 2 == 0 else nc.sync
                    out_eng.dma_start(out=y_d[:, o:o + GW], in_=ysb)

            if repeat == 1:
                one_pass()
            else:
                import concourse.mybir as _mb
                with tc.For_i(0, repeat, 1,
                              hint_engines=(_mb.EngineType.PE,),
                              staggered_reset=True):
                    for _ in range(passes_per_iter):
                        one_pass()
    nc.compile()
    return nc


def _make_in_maps(x: np.ndarray, scale: int):
    xf = np.ascontiguousarray(np.asarray(x, dtype=np.float32).reshape(-1, C))
    x16 = _f32_to_bf16_u16(xf)
    W = _build_w_chunks(scale)
    return [{"x": _shard_xt(x16, i), "w": W} for i in range(N_CORES)]


def _unshard_y(yb: np.ndarray, nrows: int) -> np.ndarray:
    """[128, N_GROUPS*GW] bf16-u16 device layout -> [nrows, C] f32."""
    v = yb.reshape(128, N_GROUPS, 6, R)         # p g v r
    v = v.transpose(1, 3, 2, 0)                 # g r v p -> rows, (v, p)
    v = v.reshape(ROWS_PER_CORE, 6 * 128)
    yf = _bf16_u16_to_f32(np.ascontiguousarray(v).view(np.uint16))
    out = np.empty((ROWS_PER_CORE, C), dtype=np.float32)
    # v blocks 0-2: even channels 2*(v*128+p); 3-5: odd channels
    out[:, 0::2] = yf[:, :H]
    out[:, 1::2] = yf[:, 384:384 + H]
    return out[:nrows]


def kernel(x: np.ndarray, scale) -> np.ndarray:
    import sys
    if "/opt/trn_rl_repo" not in sys.path:
        sys.path.insert(0, "/opt/trn_rl_repo")
    from concourse.bass_utils import run_bass_kernel_spmd

    scale = int(np.asarray(scale))
    x = np.asarray(x, dtype=np.float32)
    orig_shape = x.shape

    if "nc" not in _CACHE:
        _CACHE["nc"] = _build_nc()
    nc = _CACHE["nc"]

    in_maps = _make_in_maps(x, scale)
    res = run_bass_kernel_spmd(nc, in_maps, list(range(N_CORES)))
    outs = []
    for i, r in enumerate(res.results):
        lo = i * ROWS_PER_CORE
        hi = min(lo + ROWS_PER_CORE, ROWS_TOTAL)
        outs.append(_unshard_y(np.asarray(r["y"]), hi - lo))
    y = np.concatenate(outs, axis=0).reshape(orig_shape)
    return y.astype(np.float32)
